# revision 1
# baseline (speedup 1.0000x reference)
"""LSTM decoder kernel for Trainium2 (8 NeuronCores, data-parallel over batch).

Reference computation (per batch element b):
    h0 = context_seq[b, -1, :]          # only the LAST timestep is used
    c0 = 0
    for t in range(T):
        gates = h @ (W_ih + W_hh).T + (b_ih + b_hh)     # [4H], order i,f,g,o
        i, f, g, o = split(gates)
        c = sigmoid(f) * c + sigmoid(i) * tanh(g)
        h = sigmoid(o) * tanh(c)
        pred[t] = h @ W_out.T + b_out                   # [O]

Device layout (per core, B=128 batch rows):
    state kept TRANSPOSED: hT, cT are [H=128 partitions, B=128 free].
    gates^T computed with 4 matmuls (stationary = weight block) into one
    PSUM tile [128, 4*B]; per-gate sigmoid/tanh applied with per-partition
    bias on ScalarE; elementwise updates on VectorE; per-step prediction
    via a small matmul (stationary = hT) giving pred [B, 7] naturally,
    accumulated into an SBUF buffer [128, T*7]; one DMA out at the end.
b_out is added on the host.
"""

import json

import numpy as np

B_TOTAL = 1024
H = 128
O = 7
N_CORES = 8
B_CORE = B_TOTAL // N_CORES  # 128


def _split_multiwait(bir_bytes: bytes) -> bytes:
    """This walrus build encodes at most ONE sync-wait per instruction.
    Split any multi-wait instruction into single-wait NoOps on the same
    engine (the sequencer executes them in program order, so waiting on
    each semaphore in turn is equivalent to waiting on all of them)."""
    bir = json.loads(bir_bytes)
    n = 0
    for f in bir.get("functions", []):
        for blk in f.get("blocks", []):
            new = []
            for inst in blk.get("instructions", []):
                si = inst.get("sync_info")
                waits = (si or {}).get("on_wait") or []
                if len(waits) > 1:
                    for w in waits[:-1]:
                        n += 1
                        nop = {
                            "name": f"WSPLIT-{n}",
                            "engine": inst.get("engine"),
                            "ins": [],
                            "outs": [],
                            "opcode": "NoOp",
                            "sync_info": {"on_update": [], "on_wait": [w]},
                        }
                        if inst.get("debug") is not None:
                            nop["debug"] = inst["debug"]
                        new.append(nop)
                    si["on_wait"] = [waits[-1]]
                new.append(inst)
            blk["instructions"] = new
    return json.dumps(bir).encode()


_PATCHED = False


def _patch_bass():
    global _PATCHED
    if _PATCHED:
        return
    import concourse.bass as bass

    orig = bass.Bass.to_json_bytes

    def patched(self, *a, **k):
        return _split_multiwait(orig(self, *a, **k))

    bass.Bass.to_json_bytes = patched
    _PATCHED = True


_PROGRAM_CACHE = {}


def _build_program(T: int):
    """Build the Bass/Tile program for T recurrence steps. Returns nc."""
    import concourse.bass as bass
    import concourse.tile as tile
    from concourse import mybir

    _patch_bass()

    fp32 = mybir.dt.float32
    AF = mybir.ActivationFunctionType

    nc = bass.Bass("TRN2", debug=False)
    # DRAM I/O (per-core shard shapes)
    d_h0t = nc.dram_tensor("h0t", [H, B_CORE], fp32, kind="ExternalInput").ap()
    d_wt = nc.dram_tensor("wt", [H, 4 * H], fp32, kind="ExternalInput").ap()
    d_bias = nc.dram_tensor("bias", [H, 4], fp32, kind="ExternalInput").ap()
    d_woutt = nc.dram_tensor("woutt", [H, O], fp32, kind="ExternalInput").ap()
    d_preds = nc.dram_tensor("preds", [B_CORE, T * O], fp32, kind="ExternalOutput").ap()

    with tile.TileContext(nc) as tc:
        with (
            tc.tile_pool(name="fixed", bufs=1) as fixed,
            tc.tile_pool(name="state", bufs=2) as state,
            tc.tile_pool(name="acts", bufs=2) as acts,
            tc.tile_pool(name="psum", bufs=2, space="PSUM") as psum_pool,
            tc.tile_pool(name="ppsum", bufs=2, space="PSUM") as ppsum_pool,
            tc.tile_pool(name="outp", bufs=1) as outp,
        ):
            wt = fixed.tile([H, 4 * H], fp32)
            nc.sync.dma_start(wt[:], d_wt[:])
            bias = fixed.tile([H, 4], fp32)
            nc.sync.dma_start(bias[:], d_bias[:])
            woutt = fixed.tile([H, O], fp32)
            nc.sync.dma_start(woutt[:], d_woutt[:])

            outbuf = outp.tile([B_CORE, T * O], fp32)

            hT = state.tile([H, B_CORE], fp32, tag="h")
            nc.sync.dma_start(hT[:], d_h0t[:])
            cT = state.tile([H, B_CORE], fp32, tag="c")
            nc.vector.memset(cT[:], 0.0)

            Bc = B_CORE
            for t in range(T):
                gps = psum_pool.tile([128, 4 * Bc], fp32, tag="gpsum")
                # gates^T: out[gate_row, b] — stationary = weight block
                for g in range(4):
                    nc.tensor.matmul(
                        gps[:, g * Bc : (g + 1) * Bc],
                        wt[:, g * H : (g + 1) * H],
                        hT[:],
                        start=True,
                        stop=True,
                    )
                i_s = acts.tile([H, Bc], fp32, tag="i_s")
                nc.scalar.activation(i_s[:], gps[:, 0:Bc], AF.Sigmoid, bias=bias[:, 0:1])
                f_s = acts.tile([H, Bc], fp32, tag="f_s")
                nc.scalar.activation(f_s[:], gps[:, Bc : 2 * Bc], AF.Sigmoid, bias=bias[:, 1:2])
                g_t = acts.tile([H, Bc], fp32, tag="g_t")
                nc.scalar.activation(g_t[:], gps[:, 2 * Bc : 3 * Bc], AF.Tanh, bias=bias[:, 2:3])
                o_s = acts.tile([H, Bc], fp32, tag="o_s")
                nc.scalar.activation(o_s[:], gps[:, 3 * Bc : 4 * Bc], AF.Sigmoid, bias=bias[:, 3:4])

                t1 = acts.tile([H, Bc], fp32, tag="t1")
                nc.vector.tensor_mul(t1[:], f_s[:], cT[:])
                t2 = acts.tile([H, Bc], fp32, tag="t2")
                nc.vector.tensor_mul(t2[:], i_s[:], g_t[:])
                cT = state.tile([H, Bc], fp32, tag="c")
                nc.vector.tensor_add(cT[:], t1[:], t2[:])

                th = acts.tile([H, Bc], fp32, tag="th")
                nc.scalar.activation(th[:], cT[:], AF.Tanh)
                hT = state.tile([H, Bc], fp32, tag="h")
                nc.vector.tensor_mul(hT[:], o_s[:], th[:])

                pps = ppsum_pool.tile([Bc, O], fp32, tag="ppsum")
                nc.tensor.matmul(pps[:], hT[:], woutt[:], start=True, stop=True)
                nc.vector.tensor_copy(outbuf[:, t * O : (t + 1) * O], pps[:])

            nc.sync.dma_start(d_preds[:], outbuf[:])

    return nc


def _get_program(T: int):
    if T not in _PROGRAM_CACHE:
        _PROGRAM_CACHE[T] = _build_program(T)
    return _PROGRAM_CACHE[T]


def kernel(
    context_seq,
    W_ih,
    W_hh,
    b_ih,
    b_hh,
    W_out,
    b_out,
    prediction_len,
):
    from concourse.bass_utils import run_bass_kernel_spmd

    T = int(prediction_len)
    context_seq = np.asarray(context_seq, dtype=np.float32)
    W_ih = np.asarray(W_ih, dtype=np.float32)
    W_hh = np.asarray(W_hh, dtype=np.float32)
    b_ih = np.asarray(b_ih, dtype=np.float32)
    b_hh = np.asarray(b_hh, dtype=np.float32)
    W_out = np.asarray(W_out, dtype=np.float32)
    b_out = np.asarray(b_out, dtype=np.float32)

    B = context_seq.shape[0]
    assert B == B_TOTAL and context_seq.shape[2] == H

    # Host-side prep: only the last timestep of context_seq is used.
    h0 = context_seq[:, -1, :]  # [B, H]
    W = W_ih + W_hh  # [4H, H]
    b = b_ih + b_hh  # [4H]
    wt = np.ascontiguousarray(W.T)  # [H, 4H]; col g*H+m = W[g block row m]
    bias_cols = np.ascontiguousarray(b.reshape(4, H).T)  # [H, 4]
    woutt = np.ascontiguousarray(W_out.T)  # [H, O]

    nc = _get_program(T)

    in_maps = []
    for c in range(N_CORES):
        sh = h0[c * B_CORE : (c + 1) * B_CORE]  # [B_CORE, H]
        in_maps.append(
            {
                "h0t": np.ascontiguousarray(sh.T),  # [H, B_CORE]
                "wt": wt,
                "bias": bias_cols,
                "woutt": woutt,
            }
        )

    res = run_bass_kernel_spmd(nc, in_maps, core_ids=list(range(N_CORES)))

    out = np.empty((B_TOTAL, T, O), dtype=np.float32)
    for c in range(N_CORES):
        out[c * B_CORE : (c + 1) * B_CORE] = res.results[c]["preds"].reshape(
            B_CORE, T, O
        )
    out += b_out  # broadcast over [B, T, O]
    return out
